# revision 2
# baseline (speedup 1.0000x reference)
"""Multi-head attention (B=2, S=2048, D=1024, H=16) on 8 Trainium2 cores.

Sharding: head-pair. Core c handles heads {2c, 2c+1} over BOTH batches.
All cores receive identical q/k/v (both batches, transposed); only the
weight column slices differ. Per core:
  1. QKV projections (bf16 matmuls): qw^T/kw^T transposed-layout via
     lhsT=W_slice (natural), rhs=x^T; vw natural via lhsT=v^T blocks,
     rhs=Wv. Biases fused (DVE per-partition scalar add for qw^T/kw^T;
     K=1 ones-matmul for vw which also writes the ones columns used to
     fuse softmax-denominator sums into the PV matmul).
  2. Attention per (head, batch, q-chunk) unit: scores^T [k,q] with
     K=64 matmuls (two heads at PE row strips 0-63 / 64-127); exp on
     ACT (scale=1/8 fused, no max subtraction -- scores are N(0,1));
     PV+sums in one matmul stream via [vw|ones] lhsT; normalize with
     reciprocal_approx_fast.
  3. Two 8-core AllToAlls (one per head) exchange ctx^T; every chunk is
     useful (no double-send): dest core j receives, from each source,
     that source's head channels for j's (batch, q-slice). Head-A's
     exchange hides under head-B's compute.
  4. Output projection: full 1024-channel contraction split in two
     passes (head-A channels staged to SBUF while head-B's exchange is
     in flight) + bias, direct disjoint slice out.
Host assembles the 8 disjoint [512,1024] slices.
"""
import contextlib
import ctypes
import os
import sys
import types

import ml_dtypes
import numpy as np

for _p in ("/opt/trn_rl_repo", os.path.expanduser("~/.axon_site/_ro/trn_rl_repo")):
    if os.path.isdir(_p) and _p not in sys.path:
        sys.path.insert(0, _p)
        break


def _install_ntff_hook():
    """run_bass_kernel_spmd(trace=True) under axon imports antenv.axon_hooks,
    which this image lacks; provide it so tracing degrades gracefully."""
    if "antenv.axon_hooks" in sys.modules:
        return
    mod = types.ModuleType("antenv.axon_hooks")
    state = {"hook": None}
    mod.set_axon_ntff_profile_hook = lambda h: state.__setitem__("hook", h)
    mod.get_axon_ntff_profile_hook = lambda: state["hook"]
    sys.modules["antenv.axon_hooks"] = mod
    try:
        import antenv

        antenv.axon_hooks = mod
    except ImportError:
        pass
    so_path = "/opt/axon/libaxon_pjrt.so"
    try:
        lib = ctypes.CDLL(so_path)
        if not hasattr(lib, "axon_start_nrt_profile"):
            return
        lib.axon_start_nrt_profile.argtypes = [
            ctypes.POINTER(ctypes.c_int64), ctypes.c_size_t]
        lib.axon_start_nrt_profile.restype = ctypes.c_int64
        lib.axon_stop_nrt_profile.argtypes = [ctypes.c_char_p]
        lib.axon_stop_nrt_profile.restype = ctypes.c_int64

        @contextlib.contextmanager
        def _ctx(output_dir, device_ids):
            import jax

            jax.devices()
            if device_ids:
                ids = (ctypes.c_int64 * len(device_ids))(*device_ids)
                rc = lib.axon_start_nrt_profile(ids, len(device_ids))
            else:
                rc = lib.axon_start_nrt_profile(None, 0)
            if rc != 0:
                raise RuntimeError(f"axon_start_nrt_profile rc={rc}")
            try:
                yield
            finally:
                n = lib.axon_stop_nrt_profile(str(output_dir).encode())
                print(f"profile: {n} ntff file(s) in {output_dir}",
                      file=sys.stderr)

        state["hook"] = _ctx
    except OSError:
        pass


_install_ntff_hook()

import concourse.bacc as bacc  # noqa: E402
import concourse.mybir as mybir  # noqa: E402
import concourse.tile as tile  # noqa: E402
from concourse.bass_utils import run_bass_kernel_spmd  # noqa: E402

F32 = mybir.dt.float32
F32R = mybir.dt.float32r
BF16 = mybir.dt.bfloat16
AF = mybir.ActivationFunctionType
MUL = mybir.AluOpType.mult
ADD = mybir.AluOpType.add

N_CORES = 8
B, S, D, H, HD = 2, 2048, 1024, 16, 64
HPC = 2            # heads per core
DPC = HPC * HD     # 128 output dims per core
NCH = 4            # q chunks of 512 per batch
QW = S // NCH      # 512
KT = S // 128      # 16 k-position tiles per batch
DKT = D // 128     # 8 d_model contraction tiles
NU = B * NCH       # 8 (batch, chunk) units == 8 A2A destinations

_CACHED_NC = None


def _build():
    nc = bacc.Bacc("TRN2", target_bir_lowering=False, debug=False,
                   num_devices=N_CORES)

    # per-core inputs (SPMD program; x tensors identical on all cores)
    qT = nc.dram_tensor("qT", [D, B * S], BF16, kind="ExternalInput").ap()
    kT = nc.dram_tensor("kT", [D, B * S], BF16, kind="ExternalInput").ap()
    vT = nc.dram_tensor("vT", [D, B * S], BF16, kind="ExternalInput").ap()
    wq = nc.dram_tensor("wq", [D, DPC], BF16, kind="ExternalInput").ap()
    wk = nc.dram_tensor("wk", [D, DPC], BF16, kind="ExternalInput").ap()
    wv = nc.dram_tensor("wv", [D, DPC], BF16, kind="ExternalInput").ap()
    bq1 = nc.dram_tensor("bq1", [DPC, 1], F32, kind="ExternalInput").ap()
    bk1 = nc.dram_tensor("bk1", [DPC, 1], F32, kind="ExternalInput").ap()
    bvx = nc.dram_tensor("bvx", [1, 2 * DPC], BF16, kind="ExternalInput").ap()
    wo2 = nc.dram_tensor("wo2", [D, D], BF16, kind="ExternalInput").ap()
    bo1 = nc.dram_tensor("bo1", [1, D], BF16, kind="ExternalInput").ap()
    out = nc.dram_tensor("out", [QW, D], F32, kind="ExternalOutput").ap()

    taps = {}
    if os.environ.get("DEBUG_TAPS"):
        taps["tqwT"] = nc.dram_tensor("tqwT", [128, B, S], F32R,
                                      kind="ExternalOutput").ap()
        taps["tkwT"] = nc.dram_tensor("tkwT", [128, B, S], F32R,
                                      kind="ExternalOutput").ap()
        taps["tvwx"] = nc.dram_tensor("tvwx", [128, B * KT, 2 * DPC], BF16,
                                      kind="ExternalOutput").ap()
        taps["tcout0"] = nc.dram_tensor("tcout0", [NU * 64, QW], BF16,
                                        kind="ExternalOutput").ap()
        taps["tcout1"] = nc.dram_tensor("tcout1", [NU * 64, QW], BF16,
                                        kind="ExternalOutput").ap()

    with tile.TileContext(nc) as tc:
        with tc.tile_pool(name="xw", bufs=1) as xw, \
             tc.tile_pool(name="dram", bufs=1, space="DRAM") as dram:
            # long-lived projection outputs
            qwT = xw.tile([128, B, S], F32R, name="qwT")   # rows: 2 heads x 64
            kwT = xw.tile([128, B, S], F32R, name="kwT")
            # per (b,kt) block: [vwA64 | onesA64 | vwB64 | onesB64]
            vwx = xw.tile([128, B * KT, 2 * DPC], BF16, name="vwx")
            bq_sb = xw.tile([DPC, 1], F32, name="bq_sb")
            bk_sb = xw.tile([DPC, 1], F32, name="bk_sb")
            bvx_sb = xw.tile([1, 2 * DPC], BF16, name="bvx_sb")
            onesb = xw.tile([1, 128], BF16, name="onesb")
            bo_sb = xw.tile([1, D], BF16, name="bo_sb")
            wo_sb = xw.tile([128, 2 * (DKT // 2), D], BF16, name="wo_sb")

            ones_f = xw.tile([1, 128], F32, name="ones_f")
            nc.gpsimd.memset(ones_f[:], 1.0)
            nc.vector.tensor_copy(onesb[:], ones_f[:])
            nc.sync.dma_start(out=bq_sb[:], in_=bq1[:])
            nc.sync.dma_start(out=bk_sb[:], in_=bk1[:])
            nc.sync.dma_start(out=bvx_sb[:], in_=bvx[:])
            nc.sync.dma_start(out=bo_sb[:], in_=bo1[:])

            cin0 = dram.tile([NU * 64, QW], BF16, name="cin0")
            cout0 = dram.tile([NU * 64, QW], BF16, name="cout0")
            cin1 = dram.tile([NU * 64, QW], BF16, name="cin1")
            cout1 = dram.tile([NU * 64, QW], BF16, name="cout1")
            cins, couts = (cin0, cin1), (cout0, cout1)

            # ---- phase 1: projections ----
            with tc.tile_pool(name="wpool", bufs=1) as wp, \
                 tc.tile_pool(name="xt", bufs=4) as xtp, \
                 tc.tile_pool(name="pps", bufs=2, space="PSUM") as pps:
                wk_sb = wp.tile([128, DKT, DPC], BF16, name="wk_sb")
                wq_sb = wp.tile([128, DKT, DPC], BF16, name="wq_sb")
                wv_sb = wp.tile([128, DKT, DPC], BF16, name="wv_sb")
                # first the weights needed first, then the big wo2 prefetch
                for w_dram, w_sb in ((wk, wk_sb), (wq, wq_sb), (wv, wv_sb)):
                    nc.sync.dma_start(
                        out=w_sb[:],
                        in_=w_dram.rearrange("(k p) n -> p k n", p=128))
                nc.sync.dma_start(
                    out=wo_sb[:], in_=wo2.rearrange("(k p) n -> p k n", p=128))

                # kw^T then qw^T: [128, b, S], rows = 2 heads x 64 dims
                for x_dram, w_sb, b_sb, dstT in (
                        (kT, wk_sb, bk_sb, kwT), (qT, wq_sb, bq_sb, qwT)):
                    for u in range(NU):
                        xt = xtp.tile([128, DKT, QW], BF16, name="xt", tag="xt")
                        nc.sync.dma_start(
                            out=xt[:],
                            in_=x_dram.rearrange("(k p) n -> p k n", p=128)
                                      [:, :, u * QW:(u + 1) * QW])
                        ps = pps.tile([128, QW], F32, name="ps", tag="ps")
                        for kk in range(DKT):
                            nc.tensor.matmul(
                                ps[:], w_sb[:, kk, :], xt[:, kk, :],
                                start=(kk == 0), stop=(kk == DKT - 1))
                        b_, ch = u // NCH, u % NCH
                        nc.vector.tensor_scalar_add(
                            dstT[:, b_, ch * QW:(ch + 1) * QW],
                            ps[:], b_sb[:])

                # vw (+bias, +ones cols)
                for u in range(NU):
                    vt = xtp.tile([128, DKT, QW], BF16, name="vt", tag="xt")
                    nc.sync.dma_start(
                        out=vt[:],
                        in_=vT.rearrange("(k p) n -> p k n", p=128)
                              [:, :, u * QW:(u + 1) * QW])
                    for sb_i in range(4):
                        blk = u * 4 + sb_i
                        ps = pps.tile([128, 2 * DPC], F32, name="psv", tag="ps")
                        for kk in range(DKT):
                            nc.tensor.matmul(
                                ps[:, 0:DPC],
                                vt[:, kk, sb_i * 128:(sb_i + 1) * 128],
                                wv_sb[:, kk, :],
                                start=(kk == 0), stop=False)
                        # K=1 ones-matmul: adds bv to cols 0:128, writes 1.0
                        # into cols 128:256 (ones for the fused sums)
                        nc.tensor.matmul(ps[:], onesb[:], bvx_sb[:],
                                         start=False, stop=True)
                        dst = vwx[:, blk, :].rearrange(
                            "p (h c) -> p h c", h=HPC)
                        nc.vector.tensor_copy(
                            dst[:, :, 0:64],
                            ps[:, 0:DPC].rearrange("p (h c) -> p h c", h=HPC))
                        nc.vector.tensor_copy(
                            dst[:, :, 64:128],
                            ps[:, DPC:2 * DPC].rearrange(
                                "p (h c) -> p h c", h=HPC))

            # ---- phase 2: attention (units software-pipelined so ACT
            # never idles at unit boundaries; per-head A2A so head 0's
            # exchange hides under head 1's compute) ----
            with tc.tile_pool(name="probs", bufs=24) as prp, \
                 tc.tile_pool(name="stg", bufs=4) as stp, \
                 tc.tile_pool(name="sps", bufs=3, space="PSUM") as sps, \
                 tc.tile_pool(name="vps", bufs=2, space="PSUM") as vps:

                def emit_scores(h, u, kths):
                    b_, ch = u // NCH, u % NCH
                    prs = []
                    for kth in kths:
                        sq = sps.tile([128, 2, 512], F32, name="sq", tag="sq")
                        for j in range(2):
                            kt = 2 * kth + j
                            nc.tensor.matmul(
                                sq[:, j, :],
                                kwT[h * 64:(h + 1) * 64, b_,
                                    kt * 128:(kt + 1) * 128],
                                qwT[h * 64:(h + 1) * 64, b_,
                                    ch * QW:(ch + 1) * QW],
                                start=True, stop=True)
                        pr = prp.tile([128, 2, 512], BF16, name="pr", tag="pr")
                        nc.scalar.activation(pr[:], sq[:], AF.Exp, scale=0.125)
                        prs.append(pr)
                    return prs

                def emit_pv(h, u, prs):
                    b_ = u // NCH
                    # fused PV+sums: lhsT=[vw|ones] -> ctx rows 0:64,
                    # sums rows 64:128
                    pv = vps.tile([128, 512], F32, name="pv", tag="pv")
                    for kt in range(KT):
                        nc.tensor.matmul(
                            pv[:],
                            vwx[:, b_ * KT + kt, h * 128:(h + 1) * 128],
                            prs[kt // 2][:, kt % 2, :],
                            start=(kt == 0), stop=(kt == KT - 1))
                    # plain DVE copy shifts sums rows 64:128 down to
                    # base 0 (custom DVE ops only work at base 0)
                    smlo = stp.tile([64, 512], F32, name="smlo", tag="smlo")
                    nc.vector.tensor_copy(smlo[:], pv[64:128, :])
                    rec = stp.tile([64, 512], F32, name="rec", tag="rec")
                    nc.vector.reciprocal_approx_fast(rec[:], smlo[:])
                    stg = stp.tile([64, 512], BF16, name="stg", tag="stg")
                    nc.vector.tensor_tensor(stg[:], pv[0:64, :], rec[:], MUL)
                    nc.sync.dma_start(
                        out=cins[h][u * 64:(u + 1) * 64, :], in_=stg[:])

                def emit_a2a(h):
                    nc.gpsimd.collective_compute(
                        "AllToAll", mybir.AluOpType.bypass,
                        replica_groups=[list(range(N_CORES))],
                        ins=[cins[h][:].opt()],
                        outs=[couts[h][:].opt()])

                pend = None
                for h in range(HPC):
                    for u in range(NU):
                        prs = emit_scores(h, u, range(KT // 4))
                        if pend is not None:
                            emit_pv(*pend)
                            if pend[0] == 0 and pend[1] == NU - 1:
                                emit_a2a(0)
                        prs += emit_scores(h, u, range(KT // 4, KT // 2))
                        pend = (h, u, prs)
                emit_pv(*pend)
                emit_a2a(1)

            # ---- phase 3: output projection ----
            if taps:
                nc.sync.dma_start(out=taps["tqwT"][:], in_=qwT[:])
                nc.sync.dma_start(out=taps["tkwT"][:], in_=kwT[:])
                nc.sync.dma_start(out=taps["tvwx"][:], in_=vwx[:])
                nc.sync.dma_start(out=taps["tcout0"][:], in_=cout0[:])
                nc.sync.dma_start(out=taps["tcout1"][:], in_=cout1[:])

            with tc.tile_pool(name="op", bufs=1) as op, \
                 tc.tile_pool(name="osb", bufs=2) as osb, \
                 tc.tile_pool(name="par", bufs=8) as par, \
                 tc.tile_pool(name="ops", bufs=2, space="PSUM") as ops, \
                 tc.tile_pool(name="wps", bufs=1, space="PSUM") as wps:
                OKT = DKT // 2  # 4 contraction tiles per head-half
                gth0 = op.tile([128, OKT, QW], BF16, name="gth0")
                nc.sync.dma_start(
                    out=gth0[:], in_=cout0.rearrange("(k p) n -> p k n", p=128))
                gth1 = op.tile([128, OKT, QW], BF16, name="gth1")
                nc.sync.dma_start(
                    out=gth1[:], in_=cout1.rearrange("(k p) n -> p k n", p=128))

                # pass 1: head-0 channels -> SBUF partials (runs while
                # head-1's A2A is in flight)
                pars = {}
                for mb in range(QW // 128):
                    for nch in range(2):
                        ps = ops.tile([128, 512], F32, name="pso", tag="pso")
                        for kk in range(OKT):
                            nc.tensor.matmul(
                                ps[:],
                                gth0[:, kk, mb * 128:(mb + 1) * 128],
                                wo_sb[:, kk, nch * 512:(nch + 1) * 512],
                                start=(kk == 0), stop=(kk == OKT - 1))
                        pt = par.tile([128, 512], F32, name="pt", tag="pt")
                        pars[(mb, nch)] = pt
                        nc.vector.tensor_copy(pt[:], ps[:])
                # bridge the A2A-1 wait so pass 2 starts at full clock
                warm = wps.tile([128, 512], F32, name="warm", tag="warm")
                for i in range(80):
                    nc.tensor.matmul(warm[:], onesb[:], bo_sb[:, 0:512],
                                     start=(i == 0), stop=(i == 79))
                # pass 2: head-1 channels + bias + pass-1 partial, copy out
                for mb in range(QW // 128):
                    osb_t = osb.tile([128, D], F32, name="osb_t", tag="osb")
                    for nch in range(2):
                        ps = ops.tile([128, 512], F32, name="ps2", tag="pso")
                        for kk in range(OKT):
                            nc.tensor.matmul(
                                ps[:],
                                gth1[:, kk, mb * 128:(mb + 1) * 128],
                                wo_sb[:, OKT + kk, nch * 512:(nch + 1) * 512],
                                start=(kk == 0), stop=False)
                        nc.tensor.matmul(
                            ps[:], onesb[:],
                            bo_sb[:, nch * 512:(nch + 1) * 512],
                            start=False, stop=True)
                        nc.vector.tensor_tensor(
                            osb_t[:, nch * 512:(nch + 1) * 512],
                            ps[:], pars[(mb, nch)][:], ADD)
                    nc.sync.dma_start(
                        out=out[mb * 128:(mb + 1) * 128, :], in_=osb_t[:])

    nc.compile()
    return nc


def _get_nc():
    global _CACHED_NC
    if _CACHED_NC is None:
        _CACHED_NC = _build()
    return _CACHED_NC


def kernel(q, k, v, Wq, bq, Wk, bk, Wv, bv, Wo, bo, _return_results=False):
    q, k, v = (np.asarray(x, np.float32) for x in (q, k, v))
    Wq, bq, Wk, bk, Wv, bv, Wo, bo = (
        np.asarray(x, np.float32) for x in (Wq, bq, Wk, bk, Wv, bv, Wo, bo))

    nc = _get_nc()

    # shared across cores: x^T for both batches, permuted Wo
    qT = np.concatenate([q[0].T, q[1].T], axis=1).astype(ml_dtypes.bfloat16)
    kTf = np.concatenate([k[0].T, k[1].T], axis=1).astype(ml_dtypes.bfloat16)
    vTf = np.concatenate([v[0].T, v[1].T], axis=1).astype(ml_dtypes.bfloat16)
    r = np.arange(NU * 64)
    idxA = 128 * (r // 64) + (r % 64)       # head 2s rows of source s
    wo2 = np.vstack([Wo[idxA], Wo[idxA + 64]]).astype(ml_dtypes.bfloat16)
    bo1 = bo.reshape(1, D).astype(ml_dtypes.bfloat16)

    in_maps = []
    for c in range(N_CORES):
        cols = slice(DPC * c, DPC * (c + 1))
        in_maps.append({
            "qT": qT,
            "kT": kTf,
            "vT": vTf,
            "wq": np.ascontiguousarray(Wq[:, cols]).astype(ml_dtypes.bfloat16),
            "wk": np.ascontiguousarray(Wk[:, cols]).astype(ml_dtypes.bfloat16),
            "wv": np.ascontiguousarray(Wv[:, cols]).astype(ml_dtypes.bfloat16),
            "bq1": np.ascontiguousarray(bq[cols].reshape(DPC, 1)),
            "bk1": np.ascontiguousarray(bk[cols].reshape(DPC, 1)),
            "bvx": np.concatenate(
                [bv[cols], np.ones(DPC, np.float32)]).reshape(
                1, 2 * DPC).astype(ml_dtypes.bfloat16),
            "wo2": wo2,
            "bo1": bo1,
        })

    res = run_bass_kernel_spmd(nc, in_maps, core_ids=list(range(N_CORES)))

    full = np.empty((B, S, D), np.float32)
    for c in range(N_CORES):
        b, j = c // 4, c % 4
        full[b, j * QW:(j + 1) * QW] = res.results[c]["out"]
    if _return_results:
        return full, res
    return full


# revision 8
# speedup vs baseline: 1.1772x; 1.1772x over previous
"""Multi-head attention (B=2, S=2048, D=1024, H=16) on 8 Trainium2 cores.

Sharding: head-pair. Core c handles heads {2c, 2c+1} over BOTH batches.
All cores receive identical q/k/v (both batches, transposed); only the
weight column slices differ. Fused schedule:
  - Preamble: batch-0 projections (kw, vw, qw) with round-robin DMA rings.
  - Attention over 8 (batch, q-chunk) units; each sq tile holds BOTH
    heads' scores (two PE row strips execute concurrently); exp on ACT
    (scale=1/8 fused); PV+softmax-denominator fused via [vw|ones] lhsT.
    Batch-1 projections are drained as PE filler between sq tiles so the
    PE never idles (keeps the p-state clock at 2.4 GHz) while ACT (the
    bottleneck) streams exps.
  - PV lags one unit; head-1 PVs of the last two units are deferred to
    bridge AllToAll-0. Two 8-core single-send AllToAlls (one per head,
    512 KB each) exchange ctx^T; every chunk is useful.
  - O-proj: pass 1 (head-0 channels) accumulates in PSUM under A2A-1,
    zero-matmuls keep the PE clock warm, pass 2 adds head-1 channels +
    bias, direct disjoint slice out.
Host assembles the 8 disjoint [512,1024] slices.
"""
import contextlib
import ctypes
import os
import sys
import types
from collections import deque

import ml_dtypes
import numpy as np

for _p in ("/opt/trn_rl_repo", os.path.expanduser("~/.axon_site/_ro/trn_rl_repo")):
    if os.path.isdir(_p) and _p not in sys.path:
        sys.path.insert(0, _p)
        break


def _install_ntff_hook():
    """run_bass_kernel_spmd(trace=True) under axon imports antenv.axon_hooks,
    which this image lacks; provide it so tracing degrades gracefully."""
    if "antenv.axon_hooks" in sys.modules:
        return
    mod = types.ModuleType("antenv.axon_hooks")
    state = {"hook": None}
    mod.set_axon_ntff_profile_hook = lambda h: state.__setitem__("hook", h)
    mod.get_axon_ntff_profile_hook = lambda: state["hook"]
    sys.modules["antenv.axon_hooks"] = mod
    try:
        import antenv

        antenv.axon_hooks = mod
    except ImportError:
        pass
    so_path = "/opt/axon/libaxon_pjrt.so"
    try:
        lib = ctypes.CDLL(so_path)
        if not hasattr(lib, "axon_start_nrt_profile"):
            return
        lib.axon_start_nrt_profile.argtypes = [
            ctypes.POINTER(ctypes.c_int64), ctypes.c_size_t]
        lib.axon_start_nrt_profile.restype = ctypes.c_int64
        lib.axon_stop_nrt_profile.argtypes = [ctypes.c_char_p]
        lib.axon_stop_nrt_profile.restype = ctypes.c_int64

        @contextlib.contextmanager
        def _ctx(output_dir, device_ids):
            import jax

            jax.devices()
            if device_ids:
                ids = (ctypes.c_int64 * len(device_ids))(*device_ids)
                rc = lib.axon_start_nrt_profile(ids, len(device_ids))
            else:
                rc = lib.axon_start_nrt_profile(None, 0)
            if rc != 0:
                raise RuntimeError(f"axon_start_nrt_profile rc={rc}")
            try:
                yield
            finally:
                n = lib.axon_stop_nrt_profile(str(output_dir).encode())
                print(f"profile: {n} ntff file(s) in {output_dir}",
                      file=sys.stderr)

        state["hook"] = _ctx
    except OSError:
        pass


_install_ntff_hook()

import concourse.bacc as bacc  # noqa: E402
import concourse.mybir as mybir  # noqa: E402
import concourse.tile as tile  # noqa: E402
from concourse.bass_utils import run_bass_kernel_spmd  # noqa: E402

F32 = mybir.dt.float32
F32R = mybir.dt.float32r
BF16 = mybir.dt.bfloat16
AF = mybir.ActivationFunctionType
MUL = mybir.AluOpType.mult

N_CORES = 8
B, S, D, H, HD = 2, 2048, 1024, 16, 64
HPC = 2            # heads per core
DPC = HPC * HD     # 128 output dims per core
NCH = 4            # q chunks of 512 per batch
QW = S // NCH      # 512
KT = S // 128      # 16 k-position tiles per batch
DKT = D // 128     # 8 d_model contraction tiles
NU = B * NCH       # 8 (batch, chunk) units == 8 A2A destinations
WARM_N = 64        # zero-matmuls bridging the A2A-1 wait

_CACHED_NC = None


def _build():
    nc = bacc.Bacc("TRN2", target_bir_lowering=False, debug=False,
                   num_devices=N_CORES)

    qT = nc.dram_tensor("qT", [D, B * S], BF16, kind="ExternalInput").ap()
    kT = nc.dram_tensor("kT", [D, B * S], BF16, kind="ExternalInput").ap()
    vT = nc.dram_tensor("vT", [D, B * S], BF16, kind="ExternalInput").ap()
    wq = nc.dram_tensor("wq", [D, DPC], BF16, kind="ExternalInput").ap()
    wk = nc.dram_tensor("wk", [D, DPC], BF16, kind="ExternalInput").ap()
    wv = nc.dram_tensor("wv", [D, DPC], BF16, kind="ExternalInput").ap()
    bq1 = nc.dram_tensor("bq1", [DPC, 1], F32, kind="ExternalInput").ap()
    bk1 = nc.dram_tensor("bk1", [DPC, 1], F32, kind="ExternalInput").ap()
    bvx = nc.dram_tensor("bvx", [1, 2 * DPC], BF16, kind="ExternalInput").ap()
    wo2 = nc.dram_tensor("wo2", [D, D], BF16, kind="ExternalInput").ap()
    bo1 = nc.dram_tensor("bo1", [1, D], BF16, kind="ExternalInput").ap()
    out = nc.dram_tensor("out", [QW, D], F32, kind="ExternalOutput").ap()

    taps = {}
    if os.environ.get("DEBUG_TAPS"):
        taps["tqwT"] = nc.dram_tensor("tqwT", [128, B, S], F32R,
                                      kind="ExternalOutput").ap()
        taps["tkwT"] = nc.dram_tensor("tkwT", [128, B, S], F32R,
                                      kind="ExternalOutput").ap()
        taps["tcout0"] = nc.dram_tensor("tcout0", [NU * 64, QW], BF16,
                                        kind="ExternalOutput").ap()
        taps["tcout1"] = nc.dram_tensor("tcout1", [NU * 64, QW], BF16,
                                        kind="ExternalOutput").ap()

    with tile.TileContext(nc) as tc:
        with tc.tile_pool(name="xw", bufs=1) as xw, \
             tc.tile_pool(name="opsb", bufs=1) as opsb, \
             tc.tile_pool(name="osb", bufs=2) as osbp, \
             tc.tile_pool(name="dram", bufs=1, space="DRAM") as dram:
            qwT = xw.tile([128, B, S], F32R, name="qwT")   # rows: 2 heads x 64
            kwT = xw.tile([128, B, S], F32R, name="kwT")
            # per (b,kt) block: [vwA64 | onesA64 | vwB64 | onesB64]
            vwx = xw.tile([128, B * KT, 2 * DPC], BF16, name="vwx")
            bq_sb = xw.tile([DPC, 1], F32, name="bq_sb")
            bk_sb = xw.tile([DPC, 1], F32, name="bk_sb")
            bvx_sb = xw.tile([1, 2 * DPC], BF16, name="bvx_sb")
            onesb = xw.tile([1, 128], BF16, name="onesb")
            zerob = xw.tile([1, 512], BF16, name="zerob")
            bo_sb = xw.tile([1, D], BF16, name="bo_sb")
            wo_sb = xw.tile([128, DKT, D], BF16, name="wo_sb")
            gth0 = opsb.tile([128, DKT // 2, QW], BF16, name="gth0")
            gth1 = opsb.tile([128, DKT // 2, QW], BF16, name="gth1")

            ones_f = xw.tile([1, 128], F32, name="ones_f")
            nc.gpsimd.memset(ones_f[:], 1.0)
            nc.gpsimd.memset(zerob[:], 0.0)
            nc.vector.tensor_copy(onesb[:], ones_f[:])
            nc.sync.dma_start(out=bq_sb[:], in_=bq1[:])
            nc.sync.dma_start(out=bk_sb[:], in_=bk1[:])
            nc.sync.dma_start(out=bvx_sb[:], in_=bvx[:])
            nc.sync.dma_start(out=bo_sb[:], in_=bo1[:])

            cin0 = dram.tile([NU * 64, QW], BF16, name="cin0")
            cout0 = dram.tile([NU * 64, QW], BF16, name="cout0")
            cin1 = dram.tile([NU * 64, QW], BF16, name="cin1")
            cout1 = dram.tile([NU * 64, QW], BF16, name="cout1")
            cins, couts = (cin0, cin1), (cout0, cout1)

            # round-robin DMA issue across engine queues (sync + gpsimd)
            rings = [nc.sync, nc.gpsimd]
            ring_i = [0]

            def rdma(dst, src):
                rings[ring_i[0] % len(rings)].dma_start(out=dst, in_=src)
                ring_i[0] += 1

            with tc.tile_pool(name="wpool", bufs=1) as wp, \
                 tc.tile_pool(name="xt", bufs=4) as xtp, \
                 tc.tile_pool(name="pps", bufs=2, space="PSUM") as pps, \
                 tc.tile_pool(name="probs", bufs=34) as prp, \
                 tc.tile_pool(name="stg", bufs=3) as stp, \
                 tc.tile_pool(name="sps", bufs=2, space="PSUM") as sps, \
                 tc.tile_pool(name="vps", bufs=2, space="PSUM") as vps:
                wk_sb = wp.tile([128, DKT, DPC], BF16, name="wk_sb")
                wq_sb = wp.tile([128, DKT, DPC], BF16, name="wq_sb")
                wv_sb = wp.tile([128, DKT, DPC], BF16, name="wv_sb")
                for w_dram, w_sb in ((wk, wk_sb), (wv, wv_sb), (wq, wq_sb)):
                    rdma(w_sb[:], w_dram.rearrange("(k p) n -> p k n", p=128))

                # ---- projection work generators ----
                def kwqw_entry(x_dram, w_sb, b_sb, dstT, u):
                    xt_ref = [None]

                    def prefetch():
                        xt_ref[0] = xtp.tile([128, DKT, QW], BF16,
                                             name="xt", tag="xt")
                        rdma(xt_ref[0][:],
                             x_dram.rearrange("(k p) n -> p k n", p=128)
                                   [:, :, u * QW:(u + 1) * QW])

                    def gen():
                        ps = pps.tile([128, QW], F32, name="ps", tag="ps")
                        for kk in range(DKT):
                            nc.tensor.matmul(
                                ps[:], w_sb[:, kk, :], xt_ref[0][:, kk, :],
                                start=(kk == 0), stop=(kk == DKT - 1))
                            yield
                        b_, ch = u // NCH, u % NCH
                        nc.vector.tensor_scalar_add(
                            dstT[:, b_, ch * QW:(ch + 1) * QW], ps[:], b_sb[:])

                    return prefetch, gen

                def vw_entry(u):
                    vt_ref = [None]

                    def prefetch():
                        vt_ref[0] = xtp.tile([128, DKT, QW], BF16,
                                             name="vt", tag="xt")
                        rdma(vt_ref[0][:],
                             vT.rearrange("(k p) n -> p k n", p=128)
                               [:, :, u * QW:(u + 1) * QW])

                    def gen():
                        for sb_i in range(4):
                            blk = u * 4 + sb_i
                            ps = pps.tile([128, 2 * DPC], F32, name="psv",
                                          tag="ps")
                            for kk in range(DKT):
                                nc.tensor.matmul(
                                    ps[:, 0:DPC],
                                    vt_ref[0][:, kk,
                                              sb_i * 128:(sb_i + 1) * 128],
                                    wv_sb[:, kk, :],
                                    start=(kk == 0), stop=False)
                                yield
                            # K=1 ones-matmul: +bv on cols 0:128, writes 1.0
                            # into cols 128:256 (ones for the fused sums)
                            nc.tensor.matmul(ps[:], onesb[:], bvx_sb[:],
                                             start=False, stop=True)
                            yield
                            dst = vwx[:, blk, :].rearrange(
                                "p (h c) -> p h c", h=HPC)
                            nc.vector.tensor_copy(
                                dst[:, :, 0:64],
                                ps[:, 0:DPC].rearrange(
                                    "p (h c) -> p h c", h=HPC))
                            nc.vector.tensor_copy(
                                dst[:, :, 64:128],
                                ps[:, DPC:2 * DPC].rearrange(
                                    "p (h c) -> p h c", h=HPC))

                    return prefetch, gen

                # ---- attention emitters ----
                prs_all = {}

                def emit_sq(u, kth):
                    b_, ch = u // NCH, u % NCH
                    sq = sps.tile([128, 2, 512], F32, name="sq", tag="sq")
                    for j in range(HPC):  # j = head; two PE row strips
                        nc.tensor.matmul(
                            sq[:, j, :],
                            kwT[j * 64:(j + 1) * 64, b_,
                                kth * 128:(kth + 1) * 128],
                            qwT[j * 64:(j + 1) * 64, b_,
                                ch * QW:(ch + 1) * QW],
                            start=True, stop=True)
                    pr = prp.tile([128, 2, 512], BF16, name="pr", tag="pr")
                    nc.scalar.activation(pr[:], sq[:], AF.Exp, scale=0.125)
                    prs_all[u].append(pr)

                def pv_gen(h, u):
                    b_ = u // NCH
                    prs = prs_all[u]
                    pv = vps.tile([128, 512], F32, name="pv", tag="pv")
                    for kt in range(KT):
                        nc.tensor.matmul(
                            pv[:],
                            vwx[:, b_ * KT + kt, h * 128:(h + 1) * 128],
                            prs[kt][:, h, :],
                            start=(kt == 0), stop=(kt == KT - 1))
                        yield
                    smlo = stp.tile([64, 512], F32, name="smlo", tag="smlo")
                    nc.vector.tensor_copy(smlo[:], pv[64:128, :])
                    rec = stp.tile([64, 512], F32, name="rec", tag="rec")
                    nc.vector.reciprocal_approx_fast(rec[:], smlo[:])
                    stg = stp.tile([64, 512], BF16, name="stg", tag="stg")
                    nc.vector.tensor_tensor(stg[:], pv[0:64, :], rec[:], MUL)
                    nc.sync.dma_start(
                        out=cins[h][u * 64:(u + 1) * 64, :], in_=stg[:])

                def emit_a2a(h):
                    nc.gpsimd.collective_compute(
                        "AllToAll", mybir.AluOpType.bypass,
                        replica_groups=[list(range(N_CORES))],
                        ins=[cins[h][:].opt()],
                        outs=[couts[h][:].opt()])

                # ---- scheduler ----
                pvq = deque()
                fillq = deque()
                PRE_AHEAD = 3

                def fill_push(entry):
                    fillq.append([entry[0], entry[1], False])

                def fill_prefetch():
                    n = 0
                    for e in fillq:
                        if n >= PRE_AHEAD:
                            break
                        if not e[2]:
                            e[0]()
                            e[2] = True
                        n += 1
                    for e in vwq:
                        if n >= PRE_AHEAD:
                            break
                        if not e[2]:
                            e[0]()
                            e[2] = True
                        n += 1

                vwq = deque()

                def vw_push(entry):
                    vwq.append([entry[0], entry[1], False])

                def warm_mm():
                    ps = pps.tile([128, QW], F32, name="pwm", tag="ps")
                    nc.tensor.matmul(ps[:], onesb[:], zerob[:],
                                     start=True, stop=True)

                def pump(g):
                    try:
                        next(g)
                        return True
                    except StopIteration:
                        return False

                def pump_entry(q):
                    e = q[0]
                    if not e[2]:
                        fill_prefetch()
                        e[2] = True
                    g = e[1]
                    if not isinstance(g, types.GeneratorType):
                        g = e[1] = g()
                    if pump(g):
                        return True
                    q.popleft()
                    fill_prefetch()
                    return False

                def drain(n, allow_warm=False):
                    cnt = 0
                    while cnt < n:
                        if pvq:
                            if pump(pvq[0]):
                                cnt += 1
                            else:
                                pvq.popleft()
                        elif fillq:
                            cnt += 1 if pump_entry(fillq) else 0
                        elif vwq:
                            cnt += 1 if pump_entry(vwq) else 0
                        elif allow_warm:
                            warm_mm()
                            cnt += 1
                        else:
                            break

                # ---- preamble: batch-0 projections ----
                for u in range(NCH):
                    fill_push(kwqw_entry(kT, wk_sb, bk_sb, kwT, u))
                for u in range(NCH):
                    fill_push(vw_entry(u))
                for u in range(NCH):
                    fill_push(kwqw_entry(qT, wq_sb, bq_sb, qwT, u))
                fill_prefetch()
                while fillq:
                    drain(8)
                # wo2 prefetch on the gpsimd ring (hidden under attention)
                nc.gpsimd.dma_start(
                    out=wo_sb[:], in_=wo2.rearrange("(k p) n -> p k n", p=128))

                # batch-1 projections drain as filler inside attention:
                # kw/qw before unit NCH's scores, vw before unit NCH's PVs
                for u in range(NCH, NU):
                    fill_push(kwqw_entry(kT, wk_sb, bk_sb, kwT, u))
                for u in range(NCH, NU):
                    fill_push(kwqw_entry(qT, wq_sb, bq_sb, qwT, u))
                for u in range(NCH, NU):
                    vw_push(vw_entry(u))
                fill_prefetch()

                # ---- attention units ----
                for u in range(NU):
                    if u == NCH:
                        # safety: batch-1 kw/qw must precede these scores
                        while fillq:
                            drain(16)
                    if u >= 1:
                        if u - 1 >= NCH:
                            # safety: batch-1 vw must precede batch-1 PVs
                            while fillq or vwq:
                                drain(16)
                        pvq.append(pv_gen(0, u - 1))
                        if u - 1 < NU - 2:
                            pvq.append(pv_gen(1, u - 1))
                    prs_all[u] = []
                    for kth in range(KT):
                        emit_sq(u, kth)
                        drain(5 if (fillq or vwq) else 3,
                              allow_warm=(u >= 1))
                # epilogue: finish remaining PVs and fire the exchanges
                while pvq or fillq or vwq:
                    drain(16)
                for _ in pv_gen(0, NU - 1):
                    pass
                emit_a2a(0)
                rdma(gth0[:], cout0.rearrange("(k p) n -> p k n", p=128))
                for h_, u_ in ((1, NU - 2), (1, NU - 1)):
                    for _ in pv_gen(h_, u_):
                        pass
                emit_a2a(1)
                rdma(gth1[:], cout1.rearrange("(k p) n -> p k n", p=128))

            # ---- output projection ----
            if taps:
                nc.sync.dma_start(out=taps["tqwT"][:], in_=qwT[:])
                nc.sync.dma_start(out=taps["tkwT"][:], in_=kwT[:])
                nc.sync.dma_start(out=taps["tcout0"][:], in_=cout0[:])
                nc.sync.dma_start(out=taps["tcout1"][:], in_=cout1[:])

            with tc.tile_pool(name="ops", bufs=8, space="PSUM") as ops:
                OKT = DKT // 2  # 4 contraction tiles per head-half
                pss = {}
                # pass 1: head-0 channels (runs while A2A-1 is in flight)
                for mb in range(QW // 128):
                    for nch in range(2):
                        ps = ops.tile([128, 512], F32, name="pso", tag="pso")
                        pss[(mb, nch)] = ps
                        for kk in range(OKT):
                            nc.tensor.matmul(
                                ps[:],
                                gth0[:, kk, mb * 128:(mb + 1) * 128],
                                wo_sb[:, kk, nch * 512:(nch + 1) * 512],
                                start=(kk == 0), stop=False)
                # bridge the A2A-1 wait at full clock: accumulate zeros
                # into one open accumulator (no-op on the result)
                for _ in range(WARM_N):
                    nc.tensor.matmul(pss[(0, 0)][:], onesb[:], zerob[:],
                                     start=False, stop=False)
                # pass 2: head-1 channels + bias, then copy out
                for mb in range(QW // 128):
                    osb_t = osbp.tile([128, D], F32, name="osb_t", tag="osb")
                    for nch in range(2):
                        ps = pss[(mb, nch)]
                        for kk in range(OKT):
                            nc.tensor.matmul(
                                ps[:],
                                gth1[:, kk, mb * 128:(mb + 1) * 128],
                                wo_sb[:, OKT + kk, nch * 512:(nch + 1) * 512],
                                start=False, stop=False)
                        nc.tensor.matmul(
                            ps[:], onesb[:],
                            bo_sb[:, nch * 512:(nch + 1) * 512],
                            start=False, stop=True)
                        nc.vector.tensor_copy(
                            osb_t[:, nch * 512:(nch + 1) * 512], ps[:])
                    nc.sync.dma_start(
                        out=out[mb * 128:(mb + 1) * 128, :], in_=osb_t[:])

    nc.compile()
    return nc


def _get_nc():
    global _CACHED_NC
    if _CACHED_NC is None:
        _CACHED_NC = _build()
    return _CACHED_NC


def kernel(q, k, v, Wq, bq, Wk, bk, Wv, bv, Wo, bo, _return_results=False):
    q, k, v = (np.asarray(x, np.float32) for x in (q, k, v))
    Wq, bq, Wk, bk, Wv, bv, Wo, bo = (
        np.asarray(x, np.float32) for x in (Wq, bq, Wk, bk, Wv, bv, Wo, bo))

    nc = _get_nc()

    # shared across cores: x^T for both batches, permuted Wo
    qT = np.concatenate([q[0].T, q[1].T], axis=1).astype(ml_dtypes.bfloat16)
    kTf = np.concatenate([k[0].T, k[1].T], axis=1).astype(ml_dtypes.bfloat16)
    vTf = np.concatenate([v[0].T, v[1].T], axis=1).astype(ml_dtypes.bfloat16)
    r = np.arange(NU * 64)
    idxA = 128 * (r // 64) + (r % 64)       # head 2s rows of source s
    wo2 = np.vstack([Wo[idxA], Wo[idxA + 64]]).astype(ml_dtypes.bfloat16)
    bo1 = bo.reshape(1, D).astype(ml_dtypes.bfloat16)

    in_maps = []
    for c in range(N_CORES):
        cols = slice(DPC * c, DPC * (c + 1))
        in_maps.append({
            "qT": qT,
            "kT": kTf,
            "vT": vTf,
            "wq": np.ascontiguousarray(Wq[:, cols]).astype(ml_dtypes.bfloat16),
            "wk": np.ascontiguousarray(Wk[:, cols]).astype(ml_dtypes.bfloat16),
            "wv": np.ascontiguousarray(Wv[:, cols]).astype(ml_dtypes.bfloat16),
            "bq1": np.ascontiguousarray(bq[cols].reshape(DPC, 1)),
            "bk1": np.ascontiguousarray(bk[cols].reshape(DPC, 1)),
            "bvx": np.concatenate(
                [bv[cols], np.ones(DPC, np.float32)]).reshape(
                1, 2 * DPC).astype(ml_dtypes.bfloat16),
            "wo2": wo2,
            "bo1": bo1,
        })

    res = run_bass_kernel_spmd(nc, in_maps, core_ids=list(range(N_CORES)))

    full = np.empty((B, S, D), np.float32)
    for c in range(N_CORES):
        b, j = c // 4, c % 4
        full[b, j * QW:(j + 1) * QW] = res.results[c]["out"]
    if _return_results:
        return full, res
    return full


# revision 17
# speedup vs baseline: 1.2368x; 1.0506x over previous
"""Multi-head attention (B=2, S=2048, D=1024, H=16) on 8 Trainium2 cores.

Sharding: head-pair. Core c handles heads {2c, 2c+1} over BOTH batches.
All cores receive identical q/k/v (both batches, transposed); only the
weight column slices differ. Fused schedule:
  - Preamble: batch-0 projections (kw, vw, qw) with round-robin DMA rings.
  - Attention over 8 (batch, q-chunk) units; each sq tile holds BOTH
    heads' scores (two PE row strips execute concurrently); exp on ACT
    (scale=1/8 fused); PV+softmax-denominator fused via [vw|ones] lhsT.
    Batch-1 projections are drained as PE filler between sq tiles so the
    PE never idles (keeps the p-state clock at 2.4 GHz) while ACT (the
    bottleneck) streams exps.
  - PV lags one unit; head-1 PVs of the last two units are deferred to
    bridge AllToAll-0. Two 8-core single-send AllToAlls (one per head,
    512 KB each) exchange ctx^T; every chunk is useful.
  - O-proj: pass 1 (head-0 channels) accumulates in PSUM under A2A-1,
    zero-matmuls keep the PE clock warm, pass 2 adds head-1 channels +
    bias, direct disjoint slice out.
Host assembles the 8 disjoint [512,1024] slices.
"""
import contextlib
import ctypes
import os
import sys
import types
from collections import deque

import ml_dtypes
import numpy as np

for _p in ("/opt/trn_rl_repo", os.path.expanduser("~/.axon_site/_ro/trn_rl_repo")):
    if os.path.isdir(_p) and _p not in sys.path:
        sys.path.insert(0, _p)
        break


def _install_ntff_hook():
    """run_bass_kernel_spmd(trace=True) under axon imports antenv.axon_hooks,
    which this image lacks; provide it so tracing degrades gracefully."""
    if "antenv.axon_hooks" in sys.modules:
        return
    mod = types.ModuleType("antenv.axon_hooks")
    state = {"hook": None}
    mod.set_axon_ntff_profile_hook = lambda h: state.__setitem__("hook", h)
    mod.get_axon_ntff_profile_hook = lambda: state["hook"]
    sys.modules["antenv.axon_hooks"] = mod
    try:
        import antenv

        antenv.axon_hooks = mod
    except ImportError:
        pass
    so_path = "/opt/axon/libaxon_pjrt.so"
    try:
        lib = ctypes.CDLL(so_path)
        if not hasattr(lib, "axon_start_nrt_profile"):
            return
        lib.axon_start_nrt_profile.argtypes = [
            ctypes.POINTER(ctypes.c_int64), ctypes.c_size_t]
        lib.axon_start_nrt_profile.restype = ctypes.c_int64
        lib.axon_stop_nrt_profile.argtypes = [ctypes.c_char_p]
        lib.axon_stop_nrt_profile.restype = ctypes.c_int64

        @contextlib.contextmanager
        def _ctx(output_dir, device_ids):
            import jax

            jax.devices()
            if device_ids:
                ids = (ctypes.c_int64 * len(device_ids))(*device_ids)
                rc = lib.axon_start_nrt_profile(ids, len(device_ids))
            else:
                rc = lib.axon_start_nrt_profile(None, 0)
            if rc != 0:
                raise RuntimeError(f"axon_start_nrt_profile rc={rc}")
            try:
                yield
            finally:
                n = lib.axon_stop_nrt_profile(str(output_dir).encode())
                print(f"profile: {n} ntff file(s) in {output_dir}",
                      file=sys.stderr)

        state["hook"] = _ctx
    except OSError:
        pass


_install_ntff_hook()

import concourse.bacc as bacc  # noqa: E402
import concourse.mybir as mybir  # noqa: E402
import concourse.tile as tile  # noqa: E402
from concourse.bass_utils import run_bass_kernel_spmd  # noqa: E402

F32 = mybir.dt.float32
F32R = mybir.dt.float32r
BF16 = mybir.dt.bfloat16
AF = mybir.ActivationFunctionType
MUL = mybir.AluOpType.mult

N_CORES = 8
B, S, D, H, HD = 2, 2048, 1024, 16, 64
HPC = 2            # heads per core
DPC = HPC * HD     # 128 output dims per core
NCH = 4            # q chunks of 512 per batch
QW = S // NCH      # 512
KT = S // 128      # 16 k-position tiles per batch
DKT = D // 128     # 8 d_model contraction tiles
NU = B * NCH       # 8 (batch, chunk) units == 8 A2A destinations
WARM_N = 12        # zero-matmuls bridging the pass-1 -> pass-2 seam

_CACHED_NC = None


def _build():
    nc = bacc.Bacc("TRN2", target_bir_lowering=False, debug=False,
                   num_devices=N_CORES)

    qT = nc.dram_tensor("qT", [D, B * S], BF16, kind="ExternalInput").ap()
    kT = nc.dram_tensor("kT", [D, B * S], BF16, kind="ExternalInput").ap()
    vT = nc.dram_tensor("vT", [D, B * S], BF16, kind="ExternalInput").ap()
    wq = nc.dram_tensor("wq", [D, DPC], BF16, kind="ExternalInput").ap()
    wk = nc.dram_tensor("wk", [D, DPC], BF16, kind="ExternalInput").ap()
    wv = nc.dram_tensor("wv", [D, DPC], BF16, kind="ExternalInput").ap()
    bq1 = nc.dram_tensor("bq1", [DPC, 1], F32, kind="ExternalInput").ap()
    bk1 = nc.dram_tensor("bk1", [DPC, 1], F32, kind="ExternalInput").ap()
    bvx = nc.dram_tensor("bvx", [1, 2 * DPC], BF16, kind="ExternalInput").ap()
    wo2 = nc.dram_tensor("wo2", [D, D], BF16, kind="ExternalInput").ap()
    bo1 = nc.dram_tensor("bo1", [1, D], BF16, kind="ExternalInput").ap()
    out = nc.dram_tensor("out", [QW, D], F32, kind="ExternalOutput").ap()

    taps = {}
    if os.environ.get("DEBUG_TAPS"):
        taps["tqwT"] = nc.dram_tensor("tqwT", [128, B, S], F32R,
                                      kind="ExternalOutput").ap()
        taps["tkwT"] = nc.dram_tensor("tkwT", [128, B, S], F32R,
                                      kind="ExternalOutput").ap()
        taps["tcout0"] = nc.dram_tensor("tcout0", [NU * 64, QW], BF16,
                                        kind="ExternalOutput").ap()
        taps["tcout1"] = nc.dram_tensor("tcout1", [NU * 64, QW], BF16,
                                        kind="ExternalOutput").ap()

    with tile.TileContext(nc) as tc:
        with tc.tile_pool(name="xw", bufs=1) as xw, \
             tc.tile_pool(name="opsb", bufs=1) as opsb, \
             tc.tile_pool(name="osb", bufs=2) as osbp, \
             tc.tile_pool(name="dram", bufs=1, space="DRAM") as dram:
            qwT = xw.tile([128, B, S], F32R, name="qwT")   # rows: 2 heads x 64
            kwT = xw.tile([128, B, S], F32R, name="kwT")
            # per (b,kt) block: [vwA64 | onesA64 | vwB64 | onesB64]
            vwx = xw.tile([128, B * KT, 2 * DPC], BF16, name="vwx")
            bq_sb = xw.tile([DPC, 1], F32, name="bq_sb")
            bk_sb = xw.tile([DPC, 1], F32, name="bk_sb")
            bvx_sb = xw.tile([1, 2 * DPC], BF16, name="bvx_sb")
            onesb = xw.tile([1, 128], BF16, name="onesb")
            zerob = xw.tile([1, 512], BF16, name="zerob")
            bo_sb = xw.tile([1, D], BF16, name="bo_sb")
            wo_sb = xw.tile([128, DKT, D], BF16, name="wo_sb")
            gth0 = opsb.tile([128, DKT // 2, QW], BF16, name="gth0")
            gth1 = opsb.tile([128, DKT // 2, QW], BF16, name="gth1")

            ones_f = xw.tile([1, 128], F32, name="ones_f")
            nc.gpsimd.memset(ones_f[:], 1.0)
            nc.gpsimd.memset(zerob[:], 0.0)
            nc.vector.tensor_copy(onesb[:], ones_f[:])
            nc.gpsimd.dma_start(out=bq_sb[:], in_=bq1[:])
            nc.gpsimd.dma_start(out=bk_sb[:], in_=bk1[:])
            nc.gpsimd.dma_start(out=bvx_sb[:], in_=bvx[:])
            nc.gpsimd.dma_start(out=bo_sb[:], in_=bo1[:])

            cin0 = dram.tile([NU * 64, QW], BF16, name="cin0")
            cout0 = dram.tile([NU * 64, QW], BF16, name="cout0")
            cin1 = dram.tile([NU * 64, QW], BF16, name="cin1")
            cout1 = dram.tile([NU * 64, QW], BF16, name="cout1")
            cins, couts = (cin0, cin1), (cout0, cout1)
            ccw_in = dram.tile([8, 1], F32, name="ccw_in")
            ccw_out = dram.tile([8, 1], F32, name="ccw_out")

            # round-robin DMA issue across engine queues. Preamble: sync +
            # scalar (ACT is idle, gpsimd runs the startup protocol);
            # attention: sync + gpsimd (ACT is busy with exps).
            rings = [nc.sync, nc.scalar]
            ring_i = [0]

            def rdma(dst, src):
                rings[ring_i[0] % len(rings)].dma_start(out=dst, in_=src)
                ring_i[0] += 1

            with tc.tile_pool(name="wpool", bufs=1) as wp, \
                 tc.tile_pool(name="xt", bufs=4) as xtp, \
                 tc.tile_pool(name="pps", bufs=2, space="PSUM") as pps, \
                 tc.tile_pool(name="probs", bufs=34) as prp, \
                 tc.tile_pool(name="stg", bufs=3) as stp, \
                 tc.tile_pool(name="sps", bufs=2, space="PSUM") as sps, \
                 tc.tile_pool(name="vps", bufs=2, space="PSUM") as vps:
                wk_sb = wp.tile([128, DKT, DPC], BF16, name="wk_sb")
                wq_sb = wp.tile([128, DKT, DPC], BF16, name="wq_sb")
                wv_sb = wp.tile([128, DKT, DPC], BF16, name="wv_sb")

                # ---- projection work generators (yield ~ns of PE work) ----
                def kwqw_entry(x_dram, w_sb, b_sb, dstT, u, deadline):
                    xt_ref = [None]

                    def prefetch():
                        xt_ref[0] = xtp.tile([128, DKT, QW], BF16,
                                             name="xt", tag="xt")
                        rdma(xt_ref[0][:],
                             x_dram.rearrange("(k p) n -> p k n", p=128)
                                   [:, :, u * QW:(u + 1) * QW])

                    def gen():
                        ps = pps.tile([128, QW], F32, name="ps", tag="ps")
                        for kk in range(DKT):
                            nc.tensor.matmul(
                                ps[:], w_sb[:, kk, :], xt_ref[0][:, kk, :],
                                start=(kk == 0), stop=(kk == DKT - 1))
                            yield 220
                        b_, ch = u // NCH, u % NCH
                        nc.vector.tensor_scalar_add(
                            dstT[:, b_, ch * QW:(ch + 1) * QW], ps[:], b_sb[:])

                    return [deadline, prefetch, gen, False]

                def vw_entry(u, deadline):
                    vt_ref = [None]

                    def prefetch():
                        vt_ref[0] = xtp.tile([128, DKT, QW], BF16,
                                             name="vt", tag="xt")
                        rdma(vt_ref[0][:],
                             vT.rearrange("(k p) n -> p k n", p=128)
                               [:, :, u * QW:(u + 1) * QW])

                    def gen():
                        for sb_i in range(4):
                            blk = u * 4 + sb_i
                            ps = pps.tile([128, 2 * DPC], F32, name="psv",
                                          tag="ps")
                            for kk in range(DKT):
                                nc.tensor.matmul(
                                    ps[:, 0:DPC],
                                    vt_ref[0][:, kk,
                                              sb_i * 128:(sb_i + 1) * 128],
                                    wv_sb[:, kk, :],
                                    start=(kk == 0), stop=False)
                                yield 140
                            # K=1 ones-matmul: +bv on cols 0:128, writes 1.0
                            # into cols 128:256 (ones for the fused sums)
                            nc.tensor.matmul(ps[:], onesb[:], bvx_sb[:],
                                             start=False, stop=True)
                            yield 110
                            dst = vwx[:, blk, :].rearrange(
                                "p (h c) -> p h c", h=HPC)
                            nc.vector.tensor_copy(
                                dst[:, :, 0:64],
                                ps[:, 0:DPC].rearrange(
                                    "p (h c) -> p h c", h=HPC))
                            nc.vector.tensor_copy(
                                dst[:, :, 64:128],
                                ps[:, DPC:2 * DPC].rearrange(
                                    "p (h c) -> p h c", h=HPC))

                    return [deadline, prefetch, gen, False]

                # ---- attention emitters ----
                prs_all = {}

                def emit_sq(u, kth):
                    b_, ch = u // NCH, u % NCH
                    sq = sps.tile([128, 2, 512], F32, name="sq", tag="sq")
                    for j in range(HPC):  # j = head; two PE row strips
                        nc.tensor.matmul(
                            sq[:, j, :],
                            kwT[j * 64:(j + 1) * 64, b_,
                                kth * 128:(kth + 1) * 128],
                            qwT[j * 64:(j + 1) * 64, b_,
                                ch * QW:(ch + 1) * QW],
                            start=True, stop=True)
                    pr = prp.tile([128, 2, 512], BF16, name="pr", tag="pr")
                    nc.scalar.activation(pr[:], sq[:], AF.Exp, scale=0.125)
                    prs_all[u].append(pr)

                def pv_gen(h, u):
                    b_ = u // NCH
                    prs = prs_all[u]
                    pv = vps.tile([128, 512], F32, name="pv", tag="pv")
                    for kt in range(KT):
                        nc.tensor.matmul(
                            pv[:],
                            vwx[:, b_ * KT + kt, h * 128:(h + 1) * 128],
                            prs[kt][:, h, :],
                            start=(kt == 0), stop=(kt == KT - 1))
                        yield 220
                    smlo = stp.tile([64, 512], F32, name="smlo", tag="smlo")
                    nc.vector.tensor_copy(smlo[:], pv[64:128, :])
                    rec = stp.tile([64, 512], F32, name="rec", tag="rec")
                    nc.vector.reciprocal_approx_fast(rec[:], smlo[:])
                    stg = stp.tile([64, 512], BF16, name="stg", tag="stg")
                    nc.vector.tensor_tensor(stg[:], pv[0:64, :], rec[:], MUL)
                    nc.sync.dma_start(
                        out=cins[h][u * 64:(u + 1) * 64, :], in_=stg[:])

                def emit_a2a(h):
                    nc.gpsimd.collective_compute(
                        "AllToAll", mybir.AluOpType.bypass,
                        replica_groups=[list(range(N_CORES))],
                        ins=[cins[h][:].opt()],
                        outs=[couts[h][:].opt()])

                # ---- scheduler: deadline-ordered filler queue ----
                pvq = deque()
                fillq = deque()   # entries [deadline, prefetch, gen, started]
                PRE_AHEAD = 3

                def fill_prefetch():
                    n = 0
                    for e in fillq:
                        if n >= PRE_AHEAD:
                            break
                        if not e[3]:
                            e[1]()
                            e[3] = True
                        n += 1

                def warm_mm():
                    ps = pps.tile([128, QW], F32, name="pwm", tag="ps")
                    nc.tensor.matmul(ps[:], onesb[:], zerob[:],
                                     start=True, stop=True)

                def pump(g):
                    try:
                        return next(g)
                    except StopIteration:
                        return None

                def drain(budget, allow_warm=False):
                    spent = 0
                    while spent < budget:
                        if pvq:
                            c = pump(pvq[0])
                            if c is None:
                                pvq.popleft()
                            else:
                                spent += c
                        elif fillq:
                            e = fillq[0]
                            if not e[3]:
                                fill_prefetch()
                            g = e[2]
                            if not isinstance(g, types.GeneratorType):
                                g = e[2] = g()
                            c = pump(g)
                            if c is None:
                                fillq.popleft()
                                fill_prefetch()
                            else:
                                spent += c
                        elif allow_warm:
                            warm_mm()
                            spent += 220
                        else:
                            break

                # ---- preamble: kw(b0) + qw(b0,c0); everything else fills
                # attention slots, ordered by deadline unit ----
                for u in range(NCH):
                    fillq.append(kwqw_entry(kT, wk_sb, bk_sb, kwT, u, 0))
                fillq.append(kwqw_entry(qT, wq_sb, bq_sb, qwT, 0, 0))
                fill_prefetch()
                rdma(wk_sb[:], wk.rearrange("(k p) n -> p k n", p=128))
                rdma(wv_sb[:], wv.rearrange("(k p) n -> p k n", p=128))
                rdma(wq_sb[:], wq.rearrange("(k p) n -> p k n", p=128))
                while fillq:
                    drain(4000)
                # wo2 prefetch + a2a warmup on the gpsimd queue; switch the
                # DMA rings off the scalar engine before exps start
                rings[1] = nc.gpsimd
                nc.gpsimd.dma_start(
                    out=wo_sb[:], in_=wo2.rearrange("(k p) n -> p k n", p=128))
                nc.gpsimd.dma_start(out=ccw_in[:], in_=bq1[0:8, 0:1])

                nc.gpsimd.collective_compute(
                    "AllToAll", mybir.AluOpType.bypass,
                    replica_groups=[list(range(N_CORES))],
                    ins=[ccw_in[:].opt()], outs=[ccw_out[:].opt()])

                fillq.append(kwqw_entry(qT, wq_sb, bq_sb, qwT, 1, 1))
                for u in range(NCH):
                    fillq.append(vw_entry(u, 2))
                fillq.append(kwqw_entry(qT, wq_sb, bq_sb, qwT, 2, 2))
                fillq.append(kwqw_entry(qT, wq_sb, bq_sb, qwT, 3, 3))
                for u in range(NCH, NU):
                    fillq.append(kwqw_entry(kT, wk_sb, bk_sb, kwT, u, 4))
                fillq.append(kwqw_entry(qT, wq_sb, bq_sb, qwT, NCH, 4))
                for u in range(NCH, NU):
                    fillq.append(vw_entry(u, 5))
                for ch in range(1, NCH):
                    fillq.append(
                        kwqw_entry(qT, wq_sb, bq_sb, qwT, NCH + ch, 4 + ch))
                fill_prefetch()

                # ---- attention units ----
                for u in range(NU):
                    # checkpoint: finish fillers whose deadline has arrived
                    while fillq and fillq[0][0] <= u:
                        drain(4000, allow_warm=False)
                    if u >= 2:
                        if u == 2:
                            pvq.append(pv_gen(0, 0))
                            pvq.append(pv_gen(1, 0))
                        pvq.append(pv_gen(0, u - 1))
                        if u - 1 < NU - 2:
                            pvq.append(pv_gen(1, u - 1))
                    prs_all[u] = []
                    for kth in range(KT):
                        emit_sq(u, kth)
                        drain(880, allow_warm=True)
                # epilogue: finish remaining PVs and fire the exchanges
                while pvq or fillq:
                    drain(4000)
                for _ in pv_gen(0, NU - 1):
                    pass
                emit_a2a(0)
                rdma(gth0[:], cout0.rearrange("(k p) n -> p k n", p=128))
                for h_, u_ in ((1, NU - 2), (1, NU - 1)):
                    for _ in pv_gen(h_, u_):
                        pass
                emit_a2a(1)
                rdma(gth1[:], cout1.rearrange("(k p) n -> p k n", p=128))

            # ---- output projection ----
            if taps:
                nc.sync.dma_start(out=taps["tqwT"][:], in_=qwT[:])
                nc.sync.dma_start(out=taps["tkwT"][:], in_=kwT[:])
                nc.sync.dma_start(out=taps["tcout0"][:], in_=cout0[:])
                nc.sync.dma_start(out=taps["tcout1"][:], in_=cout1[:])

            with tc.tile_pool(name="ops", bufs=8, space="PSUM") as ops:
                OKT = DKT // 2  # 4 contraction tiles per head-half
                pss = {}
                # pass 1: head-0 channels (runs while A2A-1 is in flight)
                for mb in range(QW // 128):
                    for nch in range(2):
                        ps = ops.tile([128, 512], F32, name="pso", tag="pso")
                        pss[(mb, nch)] = ps
                        for kk in range(OKT):
                            nc.tensor.matmul(
                                ps[:],
                                gth0[:, kk, mb * 128:(mb + 1) * 128],
                                wo_sb[:, kk, nch * 512:(nch + 1) * 512],
                                start=(kk == 0), stop=False)
                # bridge the A2A-1 wait at full clock: accumulate zeros
                # into one open accumulator (no-op on the result)
                for _ in range(WARM_N):
                    nc.tensor.matmul(pss[(0, 0)][:], onesb[:], zerob[:],
                                     start=False, stop=False)
                # pass 2: head-1 channels + bias, then copy out
                for mb in range(QW // 128):
                    osb_t = osbp.tile([128, D], F32, name="osb_t", tag="osb")
                    for nch in range(2):
                        ps = pss[(mb, nch)]
                        for kk in range(OKT):
                            nc.tensor.matmul(
                                ps[:],
                                gth1[:, kk, mb * 128:(mb + 1) * 128],
                                wo_sb[:, OKT + kk, nch * 512:(nch + 1) * 512],
                                start=False, stop=False)
                        nc.tensor.matmul(
                            ps[:], onesb[:],
                            bo_sb[:, nch * 512:(nch + 1) * 512],
                            start=False, stop=True)
                        nc.vector.tensor_copy(
                            osb_t[:, nch * 512:(nch + 1) * 512], ps[:])
                    nc.sync.dma_start(
                        out=out[mb * 128:(mb + 1) * 128, :], in_=osb_t[:])

    nc.compile()
    return nc


def _get_nc():
    global _CACHED_NC
    if _CACHED_NC is None:
        _CACHED_NC = _build()
    return _CACHED_NC


def kernel(q, k, v, Wq, bq, Wk, bk, Wv, bv, Wo, bo, _return_results=False):
    q, k, v = (np.asarray(x, np.float32) for x in (q, k, v))
    Wq, bq, Wk, bk, Wv, bv, Wo, bo = (
        np.asarray(x, np.float32) for x in (Wq, bq, Wk, bk, Wv, bv, Wo, bo))

    nc = _get_nc()

    # shared across cores: x^T for both batches, permuted Wo
    qT = np.concatenate([q[0].T, q[1].T], axis=1).astype(ml_dtypes.bfloat16)
    kTf = np.concatenate([k[0].T, k[1].T], axis=1).astype(ml_dtypes.bfloat16)
    vTf = np.concatenate([v[0].T, v[1].T], axis=1).astype(ml_dtypes.bfloat16)
    r = np.arange(NU * 64)
    idxA = 128 * (r // 64) + (r % 64)       # head 2s rows of source s
    wo2 = np.vstack([Wo[idxA], Wo[idxA + 64]]).astype(ml_dtypes.bfloat16)
    bo1 = bo.reshape(1, D).astype(ml_dtypes.bfloat16)

    in_maps = []
    for c in range(N_CORES):
        cols = slice(DPC * c, DPC * (c + 1))
        in_maps.append({
            "qT": qT,
            "kT": kTf,
            "vT": vTf,
            "wq": np.ascontiguousarray(Wq[:, cols]).astype(ml_dtypes.bfloat16),
            "wk": np.ascontiguousarray(Wk[:, cols]).astype(ml_dtypes.bfloat16),
            "wv": np.ascontiguousarray(Wv[:, cols]).astype(ml_dtypes.bfloat16),
            "bq1": np.ascontiguousarray(bq[cols].reshape(DPC, 1)),
            "bk1": np.ascontiguousarray(bk[cols].reshape(DPC, 1)),
            "bvx": np.concatenate(
                [bv[cols], np.ones(DPC, np.float32)]).reshape(
                1, 2 * DPC).astype(ml_dtypes.bfloat16),
            "wo2": wo2,
            "bo1": bo1,
        })

    res = run_bass_kernel_spmd(nc, in_maps, core_ids=list(range(N_CORES)))

    full = np.empty((B, S, D), np.float32)
    for c in range(N_CORES):
        b, j = c // 4, c % 4
        full[b, j * QW:(j + 1) * QW] = res.results[c]["out"]
    if _return_results:
        return full, res
    return full


# revision 29
# speedup vs baseline: 1.2762x; 1.0319x over previous
"""Multi-head attention (B=2, S=2048, D=1024, H=16) on 8 Trainium2 cores.

Sharding: head-pair. Core c handles heads {2c, 2c+1} over BOTH batches.
All cores receive identical q/k/v (both batches, transposed); only the
weight column slices differ. Fused schedule:
  - Preamble: batch-0 projections (kw, vw, qw) with round-robin DMA rings.
  - Attention over 8 (batch, q-chunk) units; each sq tile holds BOTH
    heads' scores (two PE row strips execute concurrently); exp on ACT
    (scale=1/8 fused); PV+softmax-denominator fused via [vw|ones] lhsT.
    Batch-1 projections are drained as PE filler between sq tiles so the
    PE never idles (keeps the p-state clock at 2.4 GHz) while ACT (the
    bottleneck) streams exps.
  - PV lags one unit; head-1 PVs of the last two units are deferred to
    bridge AllToAll-0. Two 8-core single-send AllToAlls (one per head,
    512 KB each) exchange ctx^T; every chunk is useful.
  - O-proj: pass 1 (head-0 channels) accumulates in PSUM under A2A-1,
    zero-matmuls keep the PE clock warm, pass 2 adds head-1 channels +
    bias, direct disjoint slice out.
Host assembles the 8 disjoint [512,1024] slices.
"""
import contextlib
import ctypes
import os
import sys
import types
from collections import deque

import ml_dtypes
import numpy as np

for _p in ("/opt/trn_rl_repo", os.path.expanduser("~/.axon_site/_ro/trn_rl_repo")):
    if os.path.isdir(_p) and _p not in sys.path:
        sys.path.insert(0, _p)
        break


def _install_ntff_hook():
    """run_bass_kernel_spmd(trace=True) under axon imports antenv.axon_hooks,
    which this image lacks; provide it so tracing degrades gracefully."""
    if "antenv.axon_hooks" in sys.modules:
        return
    mod = types.ModuleType("antenv.axon_hooks")
    state = {"hook": None}
    mod.set_axon_ntff_profile_hook = lambda h: state.__setitem__("hook", h)
    mod.get_axon_ntff_profile_hook = lambda: state["hook"]
    sys.modules["antenv.axon_hooks"] = mod
    try:
        import antenv

        antenv.axon_hooks = mod
    except ImportError:
        pass
    so_path = "/opt/axon/libaxon_pjrt.so"
    try:
        lib = ctypes.CDLL(so_path)
        if not hasattr(lib, "axon_start_nrt_profile"):
            return
        lib.axon_start_nrt_profile.argtypes = [
            ctypes.POINTER(ctypes.c_int64), ctypes.c_size_t]
        lib.axon_start_nrt_profile.restype = ctypes.c_int64
        lib.axon_stop_nrt_profile.argtypes = [ctypes.c_char_p]
        lib.axon_stop_nrt_profile.restype = ctypes.c_int64

        @contextlib.contextmanager
        def _ctx(output_dir, device_ids):
            import jax

            jax.devices()
            if device_ids:
                ids = (ctypes.c_int64 * len(device_ids))(*device_ids)
                rc = lib.axon_start_nrt_profile(ids, len(device_ids))
            else:
                rc = lib.axon_start_nrt_profile(None, 0)
            if rc != 0:
                raise RuntimeError(f"axon_start_nrt_profile rc={rc}")
            try:
                yield
            finally:
                n = lib.axon_stop_nrt_profile(str(output_dir).encode())
                print(f"profile: {n} ntff file(s) in {output_dir}",
                      file=sys.stderr)

        state["hook"] = _ctx
    except OSError:
        pass


_install_ntff_hook()

import concourse.bacc as bacc  # noqa: E402
import concourse.mybir as mybir  # noqa: E402
import concourse.tile as tile  # noqa: E402
from concourse.bass_utils import run_bass_kernel_spmd  # noqa: E402

F32 = mybir.dt.float32
F32R = mybir.dt.float32r
BF16 = mybir.dt.bfloat16
AF = mybir.ActivationFunctionType
MUL = mybir.AluOpType.mult

N_CORES = 8
B, S, D, H, HD = 2, 2048, 1024, 16, 64
HPC = 2            # heads per core
DPC = HPC * HD     # 128 output dims per core
NCH = 4            # q chunks of 512 per batch
QW = S // NCH      # 512
KT = S // 128      # 16 k-position tiles per batch
DKT = D // 128     # 8 d_model contraction tiles
NU = B * NCH       # 8 (batch, chunk) units == 8 A2A destinations
WARM_N = 12        # zero-matmuls bridging the pass-1 -> pass-2 seam

_CACHED_NC = None


def _build():
    nc = bacc.Bacc("TRN2", target_bir_lowering=False, debug=False,
                   num_devices=N_CORES)

    # arranged as [unit, partition, k-tile, col] so each per-chunk DMA
    # reads 8KB-contiguous runs per partition (fast descriptors)
    qT = nc.dram_tensor("qT", [NU, 128, DKT, QW], BF16,
                        kind="ExternalInput").ap()
    kT = nc.dram_tensor("kT", [NU, 128, DKT, QW], BF16,
                        kind="ExternalInput").ap()
    vT = nc.dram_tensor("vT", [NU, 128, DKT, QW], BF16,
                        kind="ExternalInput").ap()
    wq = nc.dram_tensor("wq", [D, DPC], BF16, kind="ExternalInput").ap()
    wk = nc.dram_tensor("wk", [D, DPC], BF16, kind="ExternalInput").ap()
    wv = nc.dram_tensor("wv", [D, DPC], BF16, kind="ExternalInput").ap()
    bq1 = nc.dram_tensor("bq1", [DPC, 1], F32, kind="ExternalInput").ap()
    bk1 = nc.dram_tensor("bk1", [DPC, 1], F32, kind="ExternalInput").ap()
    bvx = nc.dram_tensor("bvx", [1, 2 * DPC], BF16, kind="ExternalInput").ap()
    wo2 = nc.dram_tensor("wo2", [D, D], BF16, kind="ExternalInput").ap()
    bo1 = nc.dram_tensor("bo1", [1, D], BF16, kind="ExternalInput").ap()
    out = nc.dram_tensor("out", [QW, D], F32, kind="ExternalOutput").ap()

    taps = {}
    if os.environ.get("DEBUG_TAPS"):
        taps["tqwT"] = nc.dram_tensor("tqwT", [128, B, S], F32R,
                                      kind="ExternalOutput").ap()
        taps["tkwT"] = nc.dram_tensor("tkwT", [128, B, S], F32R,
                                      kind="ExternalOutput").ap()
        taps["tcout0"] = nc.dram_tensor("tcout0", [NU * 64, QW], BF16,
                                        kind="ExternalOutput").ap()
        taps["tcout1"] = nc.dram_tensor("tcout1", [NU * 64, QW], BF16,
                                        kind="ExternalOutput").ap()

    with tile.TileContext(nc) as tc:
        with tc.tile_pool(name="xw", bufs=1) as xw, \
             tc.tile_pool(name="opsb", bufs=1) as opsb, \
             tc.tile_pool(name="osb", bufs=2) as osbp, \
             tc.tile_pool(name="dram", bufs=1, space="DRAM") as dram:
            qwT = xw.tile([128, B, S], F32R, name="qwT")   # rows: 2 heads x 64
            kwT = xw.tile([128, B, S], F32R, name="kwT")
            # per (b,kt) block: [vwA64 | onesA64 | vwB64 | onesB64]
            vwx = xw.tile([128, B * KT, 2 * DPC], BF16, name="vwx")
            bq_sb = xw.tile([DPC, 1], F32, name="bq_sb")
            bk_sb = xw.tile([DPC, 1], F32, name="bk_sb")
            bvx_sb = xw.tile([1, 2 * DPC], BF16, name="bvx_sb")
            onesb = xw.tile([1, 128], BF16, name="onesb")
            zerob = xw.tile([1, 512], BF16, name="zerob")
            bo_sb = xw.tile([1, D], BF16, name="bo_sb")
            wo_sb = xw.tile([128, DKT, D], BF16, name="wo_sb")
            gth0 = opsb.tile([128, DKT // 2, QW], BF16, name="gth0")
            gth1 = opsb.tile([128, DKT // 2, QW], BF16, name="gth1")

            ones_f = xw.tile([1, 128], F32, name="ones_f")
            nc.gpsimd.memset(ones_f[:], 1.0)
            nc.gpsimd.memset(zerob[:], 0.0)
            nc.vector.tensor_copy(onesb[:], ones_f[:])
            nc.gpsimd.dma_start(out=bq_sb[:], in_=bq1[:])
            nc.gpsimd.dma_start(out=bk_sb[:], in_=bk1[:])
            nc.gpsimd.dma_start(out=bvx_sb[:], in_=bvx[:])
            nc.gpsimd.dma_start(out=bo_sb[:], in_=bo1[:])

            cin0 = dram.tile([NU * 64, QW], BF16, name="cin0")
            cout0 = dram.tile([NU * 64, QW], BF16, name="cout0")
            cin1 = dram.tile([NU * 64, QW], BF16, name="cin1")
            cout1 = dram.tile([NU * 64, QW], BF16, name="cout1")
            cins, couts = (cin0, cin1), (cout0, cout1)
            ccw_in = dram.tile([8, 1], F32, name="ccw_in")
            ccw_out = dram.tile([8, 1], F32, name="ccw_out")

            # round-robin DMA issue across engine queues. Preamble: sync +
            # scalar (ACT is idle, gpsimd runs the startup protocol);
            # attention: sync + gpsimd (ACT is busy with exps).
            rings = [nc.sync, nc.scalar]
            ring_i = [0]

            def rdma(dst, src):
                rings[ring_i[0] % len(rings)].dma_start(out=dst, in_=src)
                ring_i[0] += 1

            with tc.tile_pool(name="wpool", bufs=1) as wp, \
                 tc.tile_pool(name="xt", bufs=4) as xtp, \
                 tc.tile_pool(name="pps", bufs=2, space="PSUM") as pps, \
                 tc.tile_pool(name="probs", bufs=35) as prp, \
                 tc.tile_pool(name="stg", bufs=3) as stp, \
                 tc.tile_pool(name="sps", bufs=2, space="PSUM") as sps, \
                 tc.tile_pool(name="vps", bufs=2, space="PSUM") as vps:
                wk_sb = wp.tile([128, DKT, DPC], BF16, name="wk_sb")
                wq_sb = wp.tile([128, DKT, DPC], BF16, name="wq_sb")
                wv_sb = wp.tile([128, DKT, DPC], BF16, name="wv_sb")

                # ---- projection work generators (yield ~ns of PE work) ----
                def kwqw_entry(x_dram, w_sb, b_sb, dstT, u, deadline):
                    b_, ch = u // NCH, u % NCH
                    xt_ref = [None]

                    def prefetch():
                        xt_ref[0] = xtp.tile([128, DKT, QW], BF16,
                                             name="xt", tag="xt")
                        rdma(xt_ref[0][:], x_dram[u])

                    def gen():
                        xt = xt_ref[0]
                        ps = pps.tile([128, QW], F32, name="ps", tag="ps")
                        for kk in range(DKT):
                            nc.tensor.matmul(
                                ps[:], w_sb[:, kk, :], xt[:, kk, :],
                                start=(kk == 0), stop=(kk == DKT - 1))
                            yield 220
                        nc.vector.tensor_scalar_add(
                            dstT[:, b_, ch * QW:(ch + 1) * QW], ps[:], b_sb[:])

                    return [deadline, prefetch, gen, False]

                def vw_entry(u, deadline):
                    vt_ref = [None]

                    def prefetch():
                        vt_ref[0] = xtp.tile([128, DKT, QW], BF16,
                                             name="vt", tag="xt")
                        rdma(vt_ref[0][:], vT[u])

                    def gen():
                        vt = vt_ref[0]
                        for sb_i in range(4):
                            blk = u * 4 + sb_i
                            col = sb_i * 128
                            ps = pps.tile([128, 2 * DPC], F32, name="psv",
                                          tag="ps")
                            for kk in range(DKT):
                                nc.tensor.matmul(
                                    ps[:, 0:DPC],
                                    vt[:, kk, col:col + 128],
                                    wv_sb[:, kk, :],
                                    start=(kk == 0), stop=False)
                                yield 140
                            # K=1 ones-matmul: +bv on cols 0:128, writes 1.0
                            # into cols 128:256 (ones for the fused sums)
                            nc.tensor.matmul(ps[:], onesb[:], bvx_sb[:],
                                             start=False, stop=True)
                            yield 110
                            dst = vwx[:, blk, :].rearrange(
                                "p (h c) -> p h c", h=HPC)
                            nc.vector.tensor_copy(
                                dst[:, :, 0:64],
                                ps[:, 0:DPC].rearrange(
                                    "p (h c) -> p h c", h=HPC))
                            nc.vector.tensor_copy(
                                dst[:, :, 64:128],
                                ps[:, DPC:2 * DPC].rearrange(
                                    "p (h c) -> p h c", h=HPC))

                    return [deadline, prefetch, gen, False]

                # ---- attention emitters ----
                prs_all = {}

                def emit_sq(u, kth):
                    b_, ch = u // NCH, u % NCH
                    sq = sps.tile([128, 2, 512], F32, name="sq", tag="sq")
                    for j in range(HPC):  # j = head; two PE row strips
                        nc.tensor.matmul(
                            sq[:, j, :],
                            kwT[j * 64:(j + 1) * 64, b_,
                                kth * 128:(kth + 1) * 128],
                            qwT[j * 64:(j + 1) * 64, b_,
                                ch * QW:(ch + 1) * QW],
                            start=True, stop=True)
                    pr = prp.tile([128, 2, 512], BF16, name="pr", tag="pr")
                    nc.scalar.activation(pr[:], sq[:], AF.Exp, scale=0.125)
                    prs_all[u].append(pr)

                def pv_gen(h, u):
                    b_ = u // NCH
                    prs = prs_all[u]
                    pv = vps.tile([128, 512], F32, name="pv", tag="pv")
                    for kt in range(KT):
                        nc.tensor.matmul(
                            pv[:],
                            vwx[:, b_ * KT + kt, h * 128:(h + 1) * 128],
                            prs[kt][:, h, :],
                            start=(kt == 0), stop=(kt == KT - 1))
                        yield 220
                    smlo = stp.tile([64, 512], F32, name="smlo", tag="smlo")
                    nc.vector.tensor_copy(smlo[:], pv[64:128, :])
                    rec = stp.tile([64, 512], F32, name="rec", tag="rec")
                    nc.vector.reciprocal_approx_fast(rec[:], smlo[:])
                    stg = stp.tile([64, 512], BF16, name="stg", tag="stg")
                    nc.vector.tensor_tensor(stg[:], pv[0:64, :], rec[:], MUL)
                    nc.sync.dma_start(
                        out=cins[h][u * 64:(u + 1) * 64, :], in_=stg[:])

                def emit_a2a(h):
                    nc.gpsimd.collective_compute(
                        "AllToAll", mybir.AluOpType.bypass,
                        replica_groups=[list(range(N_CORES))],
                        ins=[cins[h][:].opt()],
                        outs=[couts[h][:].opt()])

                # ---- scheduler: deadline-ordered filler queue ----
                pvq = deque()
                fillq = deque()   # entries [deadline, prefetch, gen, started]
                PRE_AHEAD = 3

                def fill_prefetch():
                    n = 0
                    for e in fillq:
                        if n >= PRE_AHEAD:
                            break
                        if not e[3]:
                            e[1]()
                            e[3] = True
                        n += 1

                def warm_mm():
                    ps = pps.tile([128, QW], F32, name="pwm", tag="ps")
                    nc.tensor.matmul(ps[:], onesb[:], zerob[:],
                                     start=True, stop=True)

                def pump(g):
                    try:
                        return next(g)
                    except StopIteration:
                        return None

                def pump_fill():
                    e = fillq[0]
                    if not e[3]:
                        fill_prefetch()
                    g = e[2]
                    if not isinstance(g, types.GeneratorType):
                        g = e[2] = g()
                    c = pump(g)
                    if c is None:
                        fillq.popleft()
                        fill_prefetch()
                    return c

                def drain(budget, allow_warm=False):
                    # 2:1 pv:filler interleave keeps projections flowing
                    # steadily instead of bursting at deadline checkpoints
                    spent = 0
                    tick = 0
                    while spent < budget:
                        tick += 1
                        use_fill = fillq and (not pvq or tick % 3 == 0)
                        if use_fill:
                            spent += pump_fill() or 0
                        elif pvq:
                            c = pump(pvq[0])
                            if c is None:
                                pvq.popleft()
                            else:
                                spent += c
                        elif fillq:
                            spent += pump_fill() or 0
                        elif allow_warm:
                            warm_mm()
                            spent += 220
                        else:
                            break

                # ---- preamble: kw(b0) + qw(b0,c0); everything else fills
                # attention slots, ordered by deadline unit ----
                rdma(wk_sb[:], wk.rearrange("(k p) n -> p k n", p=128))
                for u in range(NCH):
                    fillq.append(kwqw_entry(kT, wk_sb, bk_sb, kwT, u, 0))
                fillq.append(kwqw_entry(qT, wq_sb, bq_sb, qwT, 0, 0))
                fill_prefetch()
                rdma(wv_sb[:], wv.rearrange("(k p) n -> p k n", p=128))
                rdma(wq_sb[:], wq.rearrange("(k p) n -> p k n", p=128))
                while fillq:
                    drain(4000)
                # wo2 prefetch + a2a warmup on the gpsimd queue; switch the
                # DMA rings off the scalar engine before exps start
                rings[1] = nc.gpsimd
                nc.gpsimd.dma_start(
                    out=wo_sb[:], in_=wo2.rearrange("(k p) n -> p k n", p=128))
                nc.gpsimd.dma_start(out=ccw_in[:], in_=bq1[0:8, 0:1])

                nc.gpsimd.collective_compute(
                    "AllToAll", mybir.AluOpType.bypass,
                    replica_groups=[list(range(N_CORES))],
                    ins=[ccw_in[:].opt()], outs=[ccw_out[:].opt()])

                fillq.append(kwqw_entry(qT, wq_sb, bq_sb, qwT, 1, 1))
                for u in range(NCH):
                    fillq.append(vw_entry(u, 2))
                fillq.append(kwqw_entry(qT, wq_sb, bq_sb, qwT, 2, 2))
                fillq.append(kwqw_entry(qT, wq_sb, bq_sb, qwT, 3, 3))
                for u in range(NCH, NU):
                    fillq.append(kwqw_entry(kT, wk_sb, bk_sb, kwT, u, 4))
                fillq.append(kwqw_entry(qT, wq_sb, bq_sb, qwT, NCH, 4))
                for u in range(NCH, NU):
                    fillq.append(vw_entry(u, 5))
                for ch in range(1, NCH):
                    fillq.append(
                        kwqw_entry(qT, wq_sb, bq_sb, qwT, NCH + ch, 4 + ch))
                fill_prefetch()

                # ---- attention units ----
                for u in range(NU):
                    # checkpoint: finish fillers whose deadline has arrived
                    while fillq and fillq[0][0] <= u:
                        drain(4000, allow_warm=False)
                    if u >= 2:
                        if u == 2:
                            pvq.append(pv_gen(0, 0))
                            pvq.append(pv_gen(1, 0))
                        pvq.append(pv_gen(0, u - 1))
                        if u - 1 < NU - 1:
                            pvq.append(pv_gen(1, u - 1))
                    prs_all[u] = []
                    for kth in range(KT):
                        emit_sq(u, kth)
                        drain(880, allow_warm=True)
                # epilogue: finish remaining PVs and fire the exchanges
                while pvq or fillq:
                    drain(4000)
                for _ in pv_gen(0, NU - 1):
                    pass
                emit_a2a(0)
                rdma(gth0[:], cout0.rearrange("(k p) n -> p k n", p=128))
                for _ in pv_gen(1, NU - 1):
                    pass
                emit_a2a(1)
                rdma(gth1[:], cout1.rearrange("(k p) n -> p k n", p=128))

            # ---- output projection ----
            if taps:
                nc.sync.dma_start(out=taps["tqwT"][:], in_=qwT[:])
                nc.sync.dma_start(out=taps["tkwT"][:], in_=kwT[:])
                nc.sync.dma_start(out=taps["tcout0"][:], in_=cout0[:])
                nc.sync.dma_start(out=taps["tcout1"][:], in_=cout1[:])

            with tc.tile_pool(name="ops", bufs=8, space="PSUM") as ops:
                OKT = DKT // 2  # 4 contraction tiles per head-half
                pss = {}
                # leading zero-matmuls (no data deps) keep the PE clock at
                # full speed while A2A-0 is still in flight; they accumulate
                # +0 into the first pass-1 group so the result is unchanged
                ps00 = ops.tile([128, 512], F32, name="pso", tag="pso")
                pss[(0, 0)] = ps00
                nc.tensor.matmul(ps00[:], onesb[:], zerob[:],
                                 start=True, stop=False)
                for _ in range(40):
                    nc.tensor.matmul(ps00[:], onesb[:], zerob[:],
                                     start=False, stop=False)
                # pass 1: head-0 channels (overlaps A2A-1)
                for mb in range(QW // 128):
                    for nch in range(2):
                        if (mb, nch) == (0, 0):
                            ps = ps00
                        else:
                            ps = ops.tile([128, 512], F32, name="pso",
                                          tag="pso")
                            pss[(mb, nch)] = ps
                        for kk in range(OKT):
                            nc.tensor.matmul(
                                ps[:],
                                gth0[:, kk, mb * 128:(mb + 1) * 128],
                                wo_sb[:, kk, nch * 512:(nch + 1) * 512],
                                start=False if (mb, nch) == (0, 0) and kk == 0
                                else (kk == 0),
                                stop=False)
                # bridge any pass1 -> pass2 seam (A2A-1 tail) at full clock
                for _ in range(WARM_N):
                    nc.tensor.matmul(pss[(3, 1)][:], onesb[:], zerob[:],
                                     start=False, stop=False)
                # pass 2: head-1 channels + bias, then copy out
                for mb in range(QW // 128):
                    osb_t = osbp.tile([128, D], F32, name="osb_t", tag="osb")
                    for nch in range(2):
                        ps = pss[(mb, nch)]
                        for kk in range(OKT):
                            nc.tensor.matmul(
                                ps[:],
                                gth1[:, kk, mb * 128:(mb + 1) * 128],
                                wo_sb[:, OKT + kk, nch * 512:(nch + 1) * 512],
                                start=False, stop=False)
                        nc.tensor.matmul(
                            ps[:], onesb[:],
                            bo_sb[:, nch * 512:(nch + 1) * 512],
                            start=False, stop=True)
                        nc.vector.tensor_copy(
                            osb_t[:, nch * 512:(nch + 1) * 512], ps[:])
                    nc.sync.dma_start(
                        out=out[mb * 128:(mb + 1) * 128, :], in_=osb_t[:])

    nc.compile()
    return nc


def _get_nc():
    global _CACHED_NC
    if _CACHED_NC is None:
        _CACHED_NC = _build()
    return _CACHED_NC


def kernel(q, k, v, Wq, bq, Wk, bk, Wv, bv, Wo, bo, _return_results=False):
    q, k, v = (np.asarray(x, np.float32) for x in (q, k, v))
    Wq, bq, Wk, bk, Wv, bv, Wo, bo = (
        np.asarray(x, np.float32) for x in (Wq, bq, Wk, bk, Wv, bv, Wo, bo))

    nc = _get_nc()

    # shared across cores: x^T arranged [unit, p, kk, n] for fast DMA
    def arrange(x):
        xT = np.concatenate([x[0].T, x[1].T], axis=1)   # [1024, 4096]
        xA = xT.reshape(DKT, 128, B, NCH, QW).transpose(2, 3, 1, 0, 4)
        return np.ascontiguousarray(
            xA.reshape(NU, 128, DKT, QW)).astype(ml_dtypes.bfloat16)

    qT = arrange(q)
    kTf = arrange(k)
    vTf = arrange(v)
    r = np.arange(NU * 64)
    idxA = 128 * (r // 64) + (r % 64)       # head 2s rows of source s
    wo2 = np.vstack([Wo[idxA], Wo[idxA + 64]]).astype(ml_dtypes.bfloat16)
    bo1 = bo.reshape(1, D).astype(ml_dtypes.bfloat16)

    in_maps = []
    for c in range(N_CORES):
        cols = slice(DPC * c, DPC * (c + 1))
        in_maps.append({
            "qT": qT,
            "kT": kTf,
            "vT": vTf,
            "wq": np.ascontiguousarray(Wq[:, cols]).astype(ml_dtypes.bfloat16),
            "wk": np.ascontiguousarray(Wk[:, cols]).astype(ml_dtypes.bfloat16),
            "wv": np.ascontiguousarray(Wv[:, cols]).astype(ml_dtypes.bfloat16),
            "bq1": np.ascontiguousarray(bq[cols].reshape(DPC, 1)),
            "bk1": np.ascontiguousarray(bk[cols].reshape(DPC, 1)),
            "bvx": np.concatenate(
                [bv[cols], np.ones(DPC, np.float32)]).reshape(
                1, 2 * DPC).astype(ml_dtypes.bfloat16),
            "wo2": wo2,
            "bo1": bo1,
        })

    res = run_bass_kernel_spmd(nc, in_maps, core_ids=list(range(N_CORES)))

    full = np.empty((B, S, D), np.float32)
    for c in range(N_CORES):
        b, j = c // 4, c % 4
        full[b, j * QW:(j + 1) * QW] = res.results[c]["out"]
    if _return_results:
        return full, res
    return full
